# revision 1
# baseline (speedup 1.0000x reference)
"""Dense soft-MoE layer for Trainium2, expert-parallel across 8 NeuronCores.

Reference computation (T=4096 tokens, D=1024, F=4096, E=8 experts):
    gate = softmax(x @ gate_w + gate_b)                  # [T, E]
    h_e  = gelu(x @ w1[e] + b1[e])                       # [T, F]
    y_e  = h_e @ w2[e] + b2[e]                           # [T, D]
    out  = sum_e gate[:, e:e+1] * y_e                    # [T, D]

Sharding: expert-parallel — core e computes gate[:, e] * y_e for all
tokens; the host sums the 8 partial outputs. Everything on device runs
in a transposed layout (hT[f, t], yT[d, t]) so no on-device transposes
are needed: every matmul's stationary operand is a weight block and the
moving operand streams 512 tokens (N=512, one PSUM bank).

Matmul data is fp16 (cast on host) with fp32 PSUM accumulation; the
expert gate is computed on-device in a [E, tokens] layout and applied
via a rank-1 broadcast matmul. Per-core expert selection is data-driven
(a one-hot selector input) so all 8 cores run the same SPMD program.
"""
import sys

sys.path.insert(0, "/opt/trn_rl_repo")

import numpy as np

D = 1024
F = 4096
E = 8
T = 4096
P = 128
TC = 512            # token chunk
NCH = T // TC       # 8 chunks
KD = D // P         # 8 d-tiles (contraction of first matmul)
KF = F // P         # 32 f-tiles (contraction of second matmul)
ND = D // P         # 8 output d-tiles

_cache = {}


def _build(reps: int = 1, loop_n: int = 0, use_tanh: bool = True, gate_pool: bool = True):
    import contextlib
    import concourse.mybir as mybir
    import concourse.tile as tile
    from concourse import bacc

    dt = mybir.dt
    AF = mybir.ActivationFunctionType
    ALU = mybir.AluOpType

    nc = bacc.Bacc(None, target_bir_lowering=False, debug=False)

    xT = nc.dram_tensor("xT", [D, T], dt.float16, kind="ExternalInput")
    w1e = nc.dram_tensor("w1e", [D, F], dt.float16, kind="ExternalInput")
    w2e = nc.dram_tensor("w2e", [F, D], dt.float16, kind="ExternalInput")
    b1e = nc.dram_tensor("b1e", [F], dt.float32, kind="ExternalInput")
    b2e = nc.dram_tensor("b2e", [D], dt.float32, kind="ExternalInput")
    gw = nc.dram_tensor("gw", [D, E], dt.float16, kind="ExternalInput")
    # gbh holds gate_b / 2: the gate exp() is computed via tanh so it shares
    # the ACT gelu table (no per-chunk table reloads): e^x = (1+t)/(1-t),
    # t = tanh(x/2) = tanh(logits*0.5 + gate_b*0.5)
    gbh = nc.dram_tensor("gbh", [E, 1], dt.float32, kind="ExternalInput")
    # one-hot selector for this core's expert (keeps the program SPMD)
    sele = nc.dram_tensor("sele", [E, 1], dt.float16, kind="ExternalInput")
    outT = nc.dram_tensor("outT", [D, T], dt.float32, kind="ExternalOutput")

    with tile.TileContext(nc) as tc:
        with tc.tile_pool(name="weights", bufs=1) as wpool, \
             tc.tile_pool(name="consts", bufs=1) as cpool, \
             tc.tile_pool(name="xin", bufs=2) as xpool, \
             tc.tile_pool(name="hbuf", bufs=1) as hpool, \
             tc.tile_pool(name="psum", bufs=6, space="PSUM") as ppool, \
             tc.tile_pool(name="gpsum", bufs=2, space="PSUM") as gpsum, \
             tc.tile_pool(name="small", bufs=4) as spool, \
             tc.tile_pool(name="gate", bufs=2) as gatepool, \
             tc.tile_pool(name="outb", bufs=3) as opool:

            w1_re = w1e.rearrange("(k p) f -> p k f", p=P)
            w1_sb = wpool.tile([P, KD, F], dt.float16)
            for f8 in range(8):
                fs = slice(f8 * (F // 8), (f8 + 1) * (F // 8))
                nc.sync.dma_start(w1_sb[:, :, fs], w1_re[:, :, fs])
            w2_re = w2e.rearrange("(k p) d -> p k d", p=P)
            w2_sb = wpool.tile([P, KF, D], dt.float16)
            for k8 in range(4):
                ks = slice(k8 * (KF // 4), (k8 + 1) * (KF // 4))
                nc.sync.dma_start(w2_sb[:, ks, :], w2_re[:, ks, :])

            b1_sb = cpool.tile([P, KF], dt.float32)
            nc.sync.dma_start(b1_sb[:], b1e.rearrange("(f p) -> p f", p=P))
            b2_sb = cpool.tile([P, ND], dt.float32)
            nc.sync.dma_start(b2_sb[:], b2e.rearrange("(d p) -> p d", p=P))
            gw_sb = cpool.tile([P, KD, E], dt.float16)
            nc.sync.dma_start(gw_sb[:], gw.rearrange("(k p) e -> p k e", p=P))
            gbh_sb = cpool.tile([E, 1], dt.float32)
            nc.sync.dma_start(gbh_sb[:], gbh[:])
            sele_sb = cpool.tile([E, 1], dt.float16)
            nc.sync.dma_start(sele_sb[:], sele[:])
            gbf_sb = cpool.tile([E, 1], dt.float32)
            nc.vector.tensor_scalar_mul(gbf_sb[:], gbh_sb[:], 2.0)
            ones8 = cpool.tile([E, 1], dt.float16)
            nc.any.memset(ones8[:], 1.0)
            ones1 = cpool.tile([1, P], dt.float16)
            nc.any.memset(ones1[:], 1.0)

            xT_re = xT.rearrange("(k p) t -> p k t", p=P)

            loop_cm = tc.For_i(0, loop_n, 1) if loop_n else contextlib.nullcontext()
            with loop_cm:
              for _rep in range(reps):
                for c in range(NCH):
                    tsl = slice(c * TC, (c + 1) * TC)
                    x_sb = xpool.tile([P, KD, TC], dt.float16, tag="x")
                    nc.sync.dma_start(x_sb[:], xT_re[:, :, tsl])

                    # --- gate: gcol[1, TC] = softmax(x@gw+gb)[:, e]^T ---
                    gp = gpsum if gate_pool else ppool
                    gtag = "gmm" if gate_pool else "mm"
                    lg = gp.tile([E, TC], dt.float32, tag=gtag)
                    for k in range(KD):
                        nc.tensor.matmul(lg[:], gw_sb[:, k, :], x_sb[:, k, :],
                                         start=(k == 0), stop=(k == KD - 1))
                    expT = spool.tile([E, TC], dt.float16, tag="expT")
                    if use_tanh:
                        tt = spool.tile([E, TC], dt.float32, tag="gs")
                        nc.scalar.activation(tt[:], lg[:], AF.Tanh,
                                             bias=gbh_sb[:], scale=0.5)
                        bm = spool.tile([E, TC], dt.float32, tag="gs")
                        nc.vector.tensor_scalar(bm[:], tt[:], -1.0, 1.0,
                                                op0=ALU.mult, op1=ALU.add)
                        rb = spool.tile([E, TC], dt.float32, tag="gs")
                        nc.vector.reciprocal(rb[:], bm[:])
                        ap1 = spool.tile([E, TC], dt.float32, tag="gs")
                        nc.vector.tensor_scalar_add(ap1[:], tt[:], 1.0)
                        nc.vector.tensor_mul(expT[:], ap1[:], rb[:])
                    else:
                        nc.scalar.activation(expT[:], lg[:], AF.Exp,
                                             bias=gbf_sb[:])
                    den = gp.tile([1, TC], dt.float32, tag=gtag)
                    nc.tensor.matmul(den[:], ones8[:], expT[:], start=True, stop=True)
                    num = gp.tile([1, TC], dt.float32, tag=gtag)
                    nc.tensor.matmul(num[:], sele_sb[:], expT[:], start=True, stop=True)
                    rec = spool.tile([1, TC], dt.float32, tag="gs")
                    nc.vector.reciprocal(rec[:], den[:])
                    gcol = spool.tile([1, TC], dt.float16, tag="gcol")
                    nc.vector.tensor_mul(gcol[:], num[:], rec[:])
                    gbc = gp.tile([P, TC], dt.float32, tag=gtag)
                    nc.tensor.matmul(gbc[:], ones1[:], gcol[:], start=True, stop=True)
                    gate_sb = gatepool.tile([P, TC], dt.float32, tag="gate")
                    nc.vector.tensor_copy(gate_sb[:], gbc[:])

                    # --- hT[f, t] = gelu(w1^T x^T + b1) ---
                    hT = hpool.tile([P, KF, TC], dt.float16, tag="hT")
                    for f in range(KF):
                        ph = ppool.tile([P, TC], dt.float32, tag="mm")
                        for k in range(KD):
                            nc.tensor.matmul(ph[:], w1_sb[:, k, f * P:(f + 1) * P],
                                             x_sb[:, k, :],
                                             start=(k == 0), stop=(k == KD - 1))
                        nc.scalar.activation(hT[:, f, :], ph[:], AF.Gelu,
                                             bias=b1_sb[:, f:f + 1])

                    # --- yT[d, t] = w2^T hT ; out = gate * (yT + b2) ---
                    for d in range(ND):
                        py = ppool.tile([P, TC], dt.float32, tag="mm")
                        for f in range(KF):
                            nc.tensor.matmul(py[:], w2_sb[:, f, d * P:(d + 1) * P],
                                             hT[:, f, :],
                                             start=(f == 0), stop=(f == KF - 1))
                        ob = opool.tile([P, TC], dt.float32, tag="ob")
                        nc.vector.scalar_tensor_tensor(
                            ob[:], py[:], b2_sb[:, d:d + 1], gate_sb[:],
                            op0=ALU.add, op1=ALU.mult)
                        nc.sync.dma_start(outT[d * P:(d + 1) * P, tsl], ob[:])

    nc.compile()
    return nc


def kernel(inputs, gate_w, gate_b, w1, b1, w2, b2):
    from concourse.bass_utils import run_bass_kernel_spmd

    if "nc" not in _cache:
        _cache["nc"] = _build()
    nc = _cache["nc"]

    B, S, Dm = inputs.shape
    x = np.ascontiguousarray(inputs.reshape(-1, Dm))          # [T, D]
    xT16 = np.ascontiguousarray(x.T).astype(np.float16)       # [D, T]
    gw16 = np.asarray(gate_w, dtype=np.float16)
    gbh32 = np.asarray(gate_b, dtype=np.float32).reshape(E, 1) * 0.5

    in_maps = []
    for e in range(E):
        sele = np.zeros((E, 1), dtype=np.float16)
        sele[e, 0] = 1.0
        in_maps.append({
            "xT": xT16,
            "w1e": np.ascontiguousarray(w1[e]).astype(np.float16),
            "w2e": np.ascontiguousarray(w2[e]).astype(np.float16),
            "b1e": np.asarray(b1[e], dtype=np.float32),
            "b2e": np.asarray(b2[e], dtype=np.float32),
            "gw": gw16,
            "gbh": gbh32,
            "sele": sele,
        })

    res = run_bass_kernel_spmd(nc, in_maps, core_ids=list(range(E)))
    _cache["last_results"] = res

    acc = res.results[0]["outT"].astype(np.float64)
    for e in range(1, E):
        acc += res.results[e]["outT"]
    out = acc.T.astype(np.float32).reshape(B, S, Dm)
    return out



# revision 3
# speedup vs baseline: 1.3297x; 1.3297x over previous
"""Dense soft-MoE layer for Trainium2, expert-parallel across 8 NeuronCores.

Reference computation (T=4096 tokens, D=1024, F=4096, E=8 experts):
    gate = softmax(x @ gate_w + gate_b)                  # [T, E]
    h_e  = gelu(x @ w1[e] + b1[e])                       # [T, F]
    y_e  = h_e @ w2[e] + b2[e]                           # [T, D]
    out  = sum_e gate[:, e:e+1] * y_e                    # [T, D]

Sharding: expert-parallel — core e computes gate[:, e] * y_e for all
tokens; the host sums the 8 partial outputs. Everything on device runs
in a transposed layout (hT[f, t], yT[d, t]) so no on-device transposes
are needed: every matmul's stationary operand is a weight block and the
moving operand streams 512 tokens (N=512, one PSUM bank).

Matmul data is fp16 (cast on host) with fp32 PSUM accumulation; the
expert gate is computed on-device in a [E, tokens] layout and applied
via a rank-1 broadcast matmul. Per-core expert selection is data-driven
(a one-hot selector input) so all 8 cores run the same SPMD program.
"""
import sys

sys.path.insert(0, "/opt/trn_rl_repo")

import numpy as np

D = 1024
F = 4096
E = 8
T = 4096
P = 128
TC = 512            # token chunk
NCH = T // TC       # 8 chunks
KD = D // P         # 8 d-tiles (contraction of first matmul)
KF = F // P         # 32 f-tiles (contraction of second matmul)
ND = D // P         # 8 output d-tiles

_cache = {}


def _build(reps: int = 1, loop_n: int = 0, use_tanh: bool = True, gate_pool: bool = True,
           mdt_name: str = "fp16"):
    import contextlib
    import concourse.mybir as mybir
    import concourse.tile as tile
    from concourse import bacc

    dt = mybir.dt
    AF = mybir.ActivationFunctionType
    ALU = mybir.AluOpType
    mdt = {"fp16": dt.float16, "bf16": dt.bfloat16}[mdt_name]

    nc = bacc.Bacc(None, target_bir_lowering=False, debug=False)

    xT = nc.dram_tensor("xT", [D, T], mdt, kind="ExternalInput")
    w1e = nc.dram_tensor("w1e", [D, F], mdt, kind="ExternalInput")
    w2e = nc.dram_tensor("w2e", [F, D], mdt, kind="ExternalInput")
    b1e = nc.dram_tensor("b1e", [F], dt.float32, kind="ExternalInput")
    b2e = nc.dram_tensor("b2e", [D], dt.float32, kind="ExternalInput")
    gw = nc.dram_tensor("gw", [D, E], mdt, kind="ExternalInput")
    # gbh holds gate_b / 2: the gate exp() is computed via tanh so it shares
    # the ACT gelu table (no per-chunk table reloads): e^x = (1+t)/(1-t),
    # t = tanh(x/2) = tanh(logits*0.5 + gate_b*0.5)
    gbh = nc.dram_tensor("gbh", [E, 1], dt.float32, kind="ExternalInput")
    # one-hot selector for this core's expert (keeps the program SPMD)
    sele = nc.dram_tensor("sele", [E, 1], mdt, kind="ExternalInput")
    outT = nc.dram_tensor("outT", [D, T], dt.float32, kind="ExternalOutput")

    with tile.TileContext(nc) as tc:
        with tc.tile_pool(name="weights", bufs=1) as wpool, \
             tc.tile_pool(name="consts", bufs=1) as cpool, \
             tc.tile_pool(name="xin", bufs=2) as xpool, \
             tc.tile_pool(name="hbuf", bufs=1) as hpool, \
             tc.tile_pool(name="psum", bufs=6, space="PSUM") as ppool, \
             tc.tile_pool(name="gpsum", bufs=2, space="PSUM") as gpsum, \
             tc.tile_pool(name="small", bufs=4) as spool, \
             tc.tile_pool(name="gate", bufs=2) as gatepool, \
             tc.tile_pool(name="outb", bufs=3) as opool:

            w1_re = w1e.rearrange("(k p) f -> p k f", p=P)
            w1_sb = wpool.tile([P, KD, F], mdt)
            for f8 in range(8):
                fs = slice(f8 * (F // 8), (f8 + 1) * (F // 8))
                nc.sync.dma_start(w1_sb[:, :, fs], w1_re[:, :, fs])
            w2_re = w2e.rearrange("(k p) d -> p k d", p=P)
            w2_sb = wpool.tile([P, KF, D], mdt)
            for k8 in range(4):
                ks = slice(k8 * (KF // 4), (k8 + 1) * (KF // 4))
                nc.sync.dma_start(w2_sb[:, ks, :], w2_re[:, ks, :])

            b1_sb = cpool.tile([P, KF], dt.float32)
            nc.sync.dma_start(b1_sb[:], b1e.rearrange("(f p) -> p f", p=P))
            b2_sb = cpool.tile([P, ND], dt.float32)
            nc.sync.dma_start(b2_sb[:], b2e.rearrange("(d p) -> p d", p=P))
            gw_sb = cpool.tile([P, KD, E], mdt)
            nc.sync.dma_start(gw_sb[:], gw.rearrange("(k p) e -> p k e", p=P))
            gbh_sb = cpool.tile([E, 1], dt.float32)
            nc.sync.dma_start(gbh_sb[:], gbh[:])
            sele_sb = cpool.tile([E, 1], mdt)
            nc.sync.dma_start(sele_sb[:], sele[:])
            gbf_sb = cpool.tile([E, 1], dt.float32)
            nc.vector.tensor_scalar_mul(gbf_sb[:], gbh_sb[:], 2.0)
            ones8 = cpool.tile([E, 1], mdt)
            nc.any.memset(ones8[:], 1.0)
            ones1 = cpool.tile([1, P], mdt)
            nc.any.memset(ones1[:], 1.0)

            xT_re = xT.rearrange("(k p) t -> p k t", p=P)

            loop_cm = tc.For_i(0, loop_n, 1) if loop_n else contextlib.nullcontext()
            with loop_cm:
              for _rep in range(reps):
                for c in range(NCH):
                    tsl = slice(c * TC, (c + 1) * TC)
                    x_sb = xpool.tile([P, KD, TC], mdt, tag="x")
                    nc.sync.dma_start(x_sb[:], xT_re[:, :, tsl])

                    # --- gate: gcol[1, TC] = softmax(x@gw+gb)[:, e]^T ---
                    gp = gpsum if gate_pool else ppool
                    gtag = "gmm" if gate_pool else "mm"
                    lg = gp.tile([E, TC], dt.float32, tag=gtag)
                    for k in range(KD):
                        nc.tensor.matmul(lg[:], gw_sb[:, k, :], x_sb[:, k, :],
                                         start=(k == 0), stop=(k == KD - 1))
                    expT = spool.tile([E, TC], mdt, tag="expT")
                    if use_tanh:
                        tt = spool.tile([E, TC], dt.float32, tag="gs")
                        nc.scalar.activation(tt[:], lg[:], AF.Tanh,
                                             bias=gbh_sb[:], scale=0.5)
                        bm = spool.tile([E, TC], dt.float32, tag="gs")
                        nc.vector.tensor_scalar(bm[:], tt[:], -1.0, 1.0,
                                                op0=ALU.mult, op1=ALU.add)
                        rb = spool.tile([E, TC], dt.float32, tag="gs")
                        nc.vector.reciprocal(rb[:], bm[:])
                        ap1 = spool.tile([E, TC], dt.float32, tag="gs")
                        nc.vector.tensor_scalar_add(ap1[:], tt[:], 1.0)
                        nc.vector.tensor_mul(expT[:], ap1[:], rb[:])
                    else:
                        nc.scalar.activation(expT[:], lg[:], AF.Exp,
                                             bias=gbf_sb[:])
                    den = gp.tile([1, TC], dt.float32, tag=gtag)
                    nc.tensor.matmul(den[:], ones8[:], expT[:], start=True, stop=True)
                    num = gp.tile([1, TC], dt.float32, tag=gtag)
                    nc.tensor.matmul(num[:], sele_sb[:], expT[:], start=True, stop=True)
                    rec = spool.tile([1, TC], dt.float32, tag="gs")
                    nc.vector.reciprocal(rec[:], den[:])
                    gcol = spool.tile([1, TC], mdt, tag="gcol")
                    nc.vector.tensor_mul(gcol[:], num[:], rec[:])
                    gbc = gp.tile([P, TC], dt.float32, tag=gtag)
                    nc.tensor.matmul(gbc[:], ones1[:], gcol[:], start=True, stop=True)
                    gate_sb = gatepool.tile([P, TC], dt.float32, tag="gate")
                    nc.vector.tensor_copy(gate_sb[:], gbc[:])

                    # --- hT[f, t] = gelu(w1^T x^T + b1) ---
                    hT = hpool.tile([P, KF, TC], mdt, tag="hT")
                    for f in range(KF):
                        ph = ppool.tile([P, TC], dt.float32, tag="mm")
                        for k in range(KD):
                            nc.tensor.matmul(ph[:], w1_sb[:, k, f * P:(f + 1) * P],
                                             x_sb[:, k, :],
                                             start=(k == 0), stop=(k == KD - 1))
                        nc.scalar.activation(hT[:, f, :], ph[:], AF.Gelu,
                                             bias=b1_sb[:, f:f + 1])

                    # --- yT[d, t] = w2^T hT ; out = gate * (yT + b2) ---
                    for d in range(ND):
                        py = ppool.tile([P, TC], dt.float32, tag="mm")
                        for f in range(KF):
                            nc.tensor.matmul(py[:], w2_sb[:, f, d * P:(d + 1) * P],
                                             hT[:, f, :],
                                             start=(f == 0), stop=(f == KF - 1))
                        ob = opool.tile([P, TC], dt.float32, tag="ob")
                        nc.vector.scalar_tensor_tensor(
                            ob[:], py[:], b2_sb[:, d:d + 1], gate_sb[:],
                            op0=ALU.add, op1=ALU.mult)
                        nc.sync.dma_start(outT[d * P:(d + 1) * P, tsl], ob[:])

    nc.compile()
    return nc


def kernel(inputs, gate_w, gate_b, w1, b1, w2, b2):
    from concourse.bass_utils import run_bass_kernel_spmd

    if "nc" not in _cache:
        _cache["nc"] = _build()
    nc = _cache["nc"]

    B, S, Dm = inputs.shape
    x = np.ascontiguousarray(inputs.reshape(-1, Dm))          # [T, D]
    xT16 = np.ascontiguousarray(x.T).astype(np.float16)       # [D, T]
    gw16 = np.asarray(gate_w, dtype=np.float16)
    gbh32 = np.asarray(gate_b, dtype=np.float32).reshape(E, 1) * 0.5

    in_maps = []
    for e in range(E):
        sele = np.zeros((E, 1), dtype=np.float16)
        sele[e, 0] = 1.0
        in_maps.append({
            "xT": xT16,
            "w1e": np.ascontiguousarray(w1[e]).astype(np.float16),
            "w2e": np.ascontiguousarray(w2[e]).astype(np.float16),
            "b1e": np.asarray(b1[e], dtype=np.float32),
            "b2e": np.asarray(b2[e], dtype=np.float32),
            "gw": gw16,
            "gbh": gbh32,
            "sele": sele,
        })

    res = run_bass_kernel_spmd(nc, in_maps, core_ids=list(range(E)))
    _cache["last_results"] = res

    acc = res.results[0]["outT"].astype(np.float64)
    for e in range(1, E):
        acc += res.results[e]["outT"]
    out = acc.T.astype(np.float32).reshape(B, S, Dm)
    return out

